# revision 1
# baseline (speedup 1.0000x reference)
"""Trainium2 Bass kernel for nn_Attention_31396210933853.

Computation (B=32, S=4096, D=512):
    eij[b,s] = sum_d x[b,s,d]*kernel[d] + bias[s]
    a        = exp(tanh(eij)) * mask
    out[b,d] = sum_s a[b,s]*x[b,s,d] / (sum_s a[b,s] + EPS)

Key restructuring: normalization is deferred (U = sum a_raw*x and
den = sum a_raw computed in one pass, out = U/(den+EPS)), so x is read
from HBM exactly once -> memory-bound at ~358 GB/s per core.

Sharding: data-parallel over batch, 4 samples per core on 8 cores.
Per-core x layout: (BC, T, 128, JJ*512) where tile (b,t) holds s-rows
s = t*(128*JJ) + p*JJ + j at partition p, free offset j*512+d.
Default JJ=2 -> 512 KiB tiles, 16 tiles/sample.

Per-tile pipeline (engines run concurrently across tiles):
  DMA (sync/HWDGE): load x tile (per-partition 4 KiB lines)
  DVE : JJ x scalar_tensor_tensor (x*k elementwise, fused
        free-dim add-reduce via accum_out) -> eraw (128,JJ)
  GpS : + bias -> eij (tiny 2-input op, keeps DVE free)
  ACT : tanh, exp
  GpS : * mask -> a (128,JJ)
  PE  : JJ matmuls a_j^T @ x_seg_j accumulated into U psum
        (1,512) per sample + ones^T @ a -> den psum, all in
        fp32r (1 cycle/row vs 4 for fp32; adds ~1e-4 rel err)
Constants stream in via GpSimd/SWDGE DMA so the sync ring's first
dispatch is the first x tile; kernel vector is broadcast to 128
partitions during its DMA (stride-0 source AP).
Finalize (per sample, after the tile loop): den = reduce(den psum)
+ EPS, rec = 1/den on DVE, out_row[b] = U * rec, one 8 KiB DMA out.

Engine budgets per core (measured): DMA ~90us (at the ~358 GB/s
HBM-per-core roofline for 32 MiB read once), DVE ~91us, PE ~63us,
ACT ~39us, GpSimd ~43us. HW exec ~113-123us.
"""
import numpy as np

import concourse.bass as bass
import concourse.bacc as bacc
import concourse.tile as tile
from concourse import mybir
from concourse.bass_utils import run_bass_kernel_spmd

B, S, D = 32, 4096, 512
N_CORES = 8
BC = B // N_CORES        # samples per core
P = 128                  # SBUF partitions
JJ = 2                   # s-rows per partition per tile
T = S // (P * JJ)        # x tiles per sample
XBUFS = 24               # x-tile pipeline depth
EPS = 1e-7

# fp32r streams the pass-B matmul at 1 cycle/row vs 4 for fp32.
PASS_B_FP32R = True

# Set by a driver (e.g. test harness) to profile; harness-off by default.
TRACE = False
LAST_RESULTS = None

_PROGRAM_CACHE = {}


def _build_program(fp32r: bool):
    f32 = mybir.dt.float32
    f32r = mybir.dt.float32r
    FT = mybir.ActivationFunctionType
    OP = mybir.AluOpType

    nc = bacc.Bacc(
        "TRN2", target_bir_lowering=False, debug=False, num_devices=N_CORES
    )
    xdt = f32r if fp32r else f32
    x_d = nc.dram_tensor("x", [BC, T, P, JJ * D], xdt, kind="ExternalInput")
    kb_d = nc.dram_tensor("kb", [1, D], f32, kind="ExternalInput")
    bias_d = nc.dram_tensor("bias_t", [P, T * JJ], f32, kind="ExternalInput")
    mask_d = nc.dram_tensor("mask_t", [BC, P, T * JJ], f32, kind="ExternalInput")
    ones_d = nc.dram_tensor("ones", [P, 1], xdt, kind="ExternalInput")
    out_d = nc.dram_tensor("out", [1, BC * D], f32, kind="ExternalOutput")

    with tile.TileContext(nc) as tc:
        with (
            tc.tile_pool(name="xp", bufs=XBUFS) as xp,
            tc.tile_pool(name="cons", bufs=1) as cons,
            tc.tile_pool(name="tmpp", bufs=6) as tmpp,
            tc.tile_pool(name="small", bufs=24) as small,
            tc.tile_pool(name="fin", bufs=4) as fin,
            tc.tile_pool(name="psum", bufs=1, space="PSUM") as psp,
        ):
            kb = cons.tile([P, D], f32)
            nc.gpsimd.dma_start(out=kb, in_=kb_d.ap().to_broadcast([P, D]))
            bias_t = cons.tile([P, T * JJ], f32)
            nc.gpsimd.dma_start(out=bias_t, in_=bias_d[:])
            mask_all = cons.tile([P, BC * T * JJ], f32)
            for b in range(BC):
                nc.gpsimd.dma_start(
                    out=mask_all[:, b * T * JJ : (b + 1) * T * JJ],
                    in_=mask_d[b],
                )
            ones = cons.tile([P, 1], xdt)
            nc.gpsimd.dma_start(out=ones, in_=ones_d[:])
            out_row = cons.tile([1, BC * D], f32)

            u_ps = [
                psp.tile([1, D], f32, name=f"u_ps{b}", tag=f"u{b}")
                for b in range(BC)
            ]
            den_ps = psp.tile([1, BC * JJ], f32, tag="den")

            finalized = set()

            def _finalize(b):
                if b in finalized:
                    return
                finalized.add(b)
                denr = fin.tile([1, 1], f32, tag="denr", name=f"denr{b}")
                nc.vector.tensor_reduce(
                    out=denr,
                    in_=den_ps[:, b * JJ : (b + 1) * JJ],
                    axis=mybir.AxisListType.X,
                    op=OP.add,
                )
                deno = fin.tile([1, 1], f32, tag="deno", name=f"deno{b}")
                nc.vector.tensor_scalar_add(deno, denr, EPS)
                rec = fin.tile([1, 1], f32, tag="rec", name=f"rec{b}")
                nc.vector.reciprocal(rec, deno)
                nc.vector.tensor_scalar_mul(
                    out_row[:, b * D : (b + 1) * D], u_ps[b], rec
                )

            for b in range(BC):
                for t in range(T):
                    x_t = xp.tile([P, JJ * D], xdt)
                    x_tf = x_t.bitcast(f32) if fp32r else x_t
                    nc.sync.dma_start(out=x_t, in_=x_d[b, t])

                    tmp = tmpp.tile([P, D], f32)
                    eraw = small.tile([P, JJ], f32)
                    for j in range(JJ):
                        nc.vector.scalar_tensor_tensor(
                            out=tmp,
                            in0=x_tf[:, j * D : (j + 1) * D],
                            scalar=0.0,
                            in1=kb,
                            op0=OP.bypass,
                            op1=OP.mult,
                            accum_out=eraw[:, j : j + 1],
                        )

                    eij = small.tile([P, JJ], f32)
                    nc.gpsimd.tensor_add(
                        eij, eraw, bias_t[:, t * JJ : (t + 1) * JJ]
                    )
                    th = small.tile([P, JJ], f32)
                    nc.scalar.activation(th, eij, FT.Tanh)
                    ex = small.tile([P, JJ], f32)
                    nc.scalar.activation(ex, th, FT.Exp)
                    a_t = small.tile([P, JJ], xdt)
                    nc.gpsimd.tensor_mul(
                        a_t, ex, mask_all[:, b * T * JJ + t * JJ : b * T * JJ + (t + 1) * JJ]
                    )

                    for j in range(JJ):
                        lhs = a_t[:, j : j + 1]
                        rhs = x_t[:, j * D : (j + 1) * D]
                        nc.tensor.matmul(
                            u_ps[b][:, :],
                            lhsT=lhs,
                            rhs=rhs,
                            start=(t == 0 and j == 0),
                            stop=(t == T - 1 and j == JJ - 1),
                        )
                    nc.tensor.matmul(
                        den_ps[:, b * JJ : (b + 1) * JJ],
                        lhsT=ones,
                        rhs=a_t,
                        start=(t == 0),
                        stop=(t == T - 1),
                    )

            for b in range(BC):
                _finalize(b)

            nc.sync.dma_start(out=out_d[:], in_=out_row)

    nc.compile()
    return nc


def _get_program(fp32r: bool):
    if fp32r not in _PROGRAM_CACHE:
        _PROGRAM_CACHE[fp32r] = _build_program(fp32r)
    return _PROGRAM_CACHE[fp32r]


def _prep_inputs(x, kern, bias, mask):
    """Host-side sharding/layout marshaling (views + tiny transposes only)."""
    x = np.ascontiguousarray(x, dtype=np.float32)
    kern = np.asarray(kern, dtype=np.float32)
    bias = np.asarray(bias, dtype=np.float32)
    kb = np.ascontiguousarray(kern[None, :])
    bias_t = np.ascontiguousarray(
        bias.reshape(T, P, JJ).transpose(1, 0, 2).reshape(P, T * JJ)
    )
    mask_f = np.asarray(mask).astype(np.float32)
    in_maps = []
    for i in range(N_CORES):
        xs = x[i * BC : (i + 1) * BC].reshape(BC, T, P, JJ * D)
        ms = (
            mask_f[i * BC : (i + 1) * BC]
            .reshape(BC, T, P, JJ)
            .transpose(0, 2, 1, 3)
            .reshape(BC, P, T * JJ)
        )
        in_maps.append(
            {
                "x": xs,
                "kb": kb,
                "bias_t": bias_t,
                "mask_t": np.ascontiguousarray(ms),
                "ones": np.ones((P, 1), dtype=np.float32),
            }
        )
    return in_maps


def kernel(x, kernel, bias, mask):
    global LAST_RESULTS
    nc = _get_program(PASS_B_FP32R)
    in_maps = _prep_inputs(x, kernel, bias, mask)
    res = run_bass_kernel_spmd(nc, in_maps, list(range(N_CORES)), trace=TRACE)
    LAST_RESULTS = res
    out = np.concatenate(
        [res.results[i]["out"].reshape(BC, D) for i in range(N_CORES)], axis=0
    )
    return out.astype(np.float32, copy=False)

